# revision 6
# baseline (speedup 1.0000x reference)
"""Trainium2 Bass kernel for nn_DifferentiableSimulator.

Math: Euler integration of 1D kinematics is a constant-coefficient linear
recurrence  v' = A*v + B*F  (A = 1 - c*dt/m_safe, B = dt/m_safe) followed by
a cumulative sum  x' = x + dt*v'.

Sequence-parallel strategy over 8 cores (T split into 8 chunks of C=1M):
 - Host (float64): the 8 chunk-boundary states (v_in, x_in per core) via
   geometric-weight dot products — O(T) vectorized numpy, exact enough.
 - Device (per core): blocked associative scan. The chunk is cut into 8192
   tiles of 128 steps. Each tile's entire scan is ONE 128x128->[128,256]
   matmul: lhsT = Xc_block (row 0 = the tile's incoming state c_k, rows
   1..127 = forces), rhs = [Mv | Mx] constant weight matrices. Tile carries
   c_k come from a cheap hierarchical scan (per-tile weighted sums via tiny
   matmuls, 128-wide column scan via triangular matmul, 64-block row scan
   via log-step), and per-tile x offsets are added for free as the
   per-partition bias of the ScalarE activation that drains PSUM.
Everything ends in an interleaved [x, v] SBUF layout that DMAs out with 1KB
contiguous bursts.
"""
import numpy as np
import concourse.bass as bass
import concourse.mybir as mybir
import concourse.tile as tile
from concourse.bass_utils import run_bass_kernel_spmd

DT = 0.01
T = 8_388_608
NCORES = 8
C = T // NCORES          # 1,048,576 timesteps per core
NB = 64                  # blocks per core (each block = 128 tiles = 16384 steps)
TS = 128                 # tile size (steps per tile) == partitions
NGRP = 8                 # DMA pipeline groups
BPG = NB // NGRP         # blocks per group
F32 = mybir.dt.float32

USE_F32R = False          # main matmuls in float32r (4x faster PE, validated)


# ---------------------------------------------------------------- host math

def _host_carries(F64, v0, x0, A, B):
    v_in = np.zeros(NCORES + 1)
    x_in = np.zeros(NCORES + 1)
    v_in[0] = v0
    x_in[0] = x0
    j = np.arange(C, dtype=np.float64)
    w_v = A ** (C - 1 - j)
    if abs(1.0 - A) > 1e-12:
        S = (1.0 - A ** (C - j)) / (1.0 - A)
        G = DT * A * (1.0 - A ** C) / (1.0 - A)
    else:
        S = C - j
        G = DT * C
    RC = A ** C
    for d in range(NCORES):
        Fc = F64[d * C:(d + 1) * C]
        x_in[d + 1] = x_in[d] + v_in[d] * G + DT * B * np.dot(S, Fc)
        v_in[d + 1] = RC * v_in[d] + B * np.dot(w_v, Fc)
    return v_in, x_in


def _device_consts(A, B):
    """Constant matrices (float64 host math, shipped as fp32)."""
    i = np.arange(TS)
    s = np.arange(TS - 1)[:, None]
    e = i[None, :] - 1 - s
    Mv = np.zeros((TS, TS))
    Mv[0, :] = A ** i
    Mv[1:, :] = np.where(e >= 0, B * A ** np.maximum(e, 0), 0.0)
    ap = A ** np.arange(TS + 1)
    cum = np.cumsum(ap)
    Mx = np.zeros((TS, TS))
    Mx[0, :] = DT * (cum[i] - 1.0)
    Mx[1:, :] = np.where(e >= 0, DT * B * cum[np.maximum(e, 0)], 0.0)
    jj = np.arange(TS)
    wlam = np.where(jj >= 1, B * A ** (128 - jj), 0.0)
    w3w = np.where(jj >= 1, B * cum[127 - jj], 0.0)
    wcat = np.stack([wlam, w3w], axis=1)
    R = A ** TS
    q = np.arange(TS)[:, None]
    m = np.arange(TS)[None, :]
    TriRex = np.where(q <= m - 1, R ** np.maximum(m - 1 - q, 0), 0.0)
    TriOnesEx = np.where(q <= m - 1, 1.0, 0.0)
    Rcol = (R ** (127 - np.arange(TS)))[:, None]
    OnesCol = np.ones((TS, 1))
    Rprow = (R ** np.arange(TS))[None, :]
    OnesRow = np.ones((1, TS))
    Bsub = np.zeros((TS, TS))
    Bsub[np.arange(1, TS), np.arange(TS - 1)] = B
    Be127 = np.zeros((1, TS))
    Be127[0, 127] = B
    Ident = np.eye(TS)
    cst = dict(mcat=np.concatenate([Mv, Mx], axis=1), wcat=wcat,
               trirex=TriRex, trione=TriOnesEx, rcol=Rcol, onescol=OnesCol,
               rprow=Rprow, onesrow=OnesRow, bsub=Bsub, be127=Be127,
               ident=Ident)
    cst = {k: np.ascontiguousarray(v, np.float32) for k, v in cst.items()}
    scal = dict(R=R, R2=R ** TS, gp=float(cum[127]))
    return cst, scal


# ---------------------------------------------------------------- bass build

def _split_multiwaits(nc, maxw=1):
    """The walrus build in this container accepts at most ONE sync wait per
    instruction (Tile's scheduler happily attaches several). Keep the first
    wait on the instruction; move extras onto EventSemaphore instructions
    inserted immediately before, same engine."""
    n = 0
    for fn in nc.m.functions:
        for bb in fn.blocks:
            new_insts = []
            changed = False
            for inst in bb.instructions:
                si = inst.sync_info
                waits = list(si.on_wait) if si and si.on_wait else []
                if len(waits) > maxw:
                    changed = True
                    for w in waits[maxw:]:
                        ev = mybir.InstEventSemaphore(
                            name=f"{inst.name}-ws{n}", engine=inst.engine,
                            ins=[], outs=[],
                            sync_info=mybir.SyncInfo(on_wait=[w], on_update=[]),
                        )
                        new_insts.append(ev)
                        n += 1
                    si.on_wait = waits[:maxw]
                new_insts.append(inst)
            if changed:
                bb.instructions = new_insts
    return n


def _build_nc(scal, use_f32r=USE_F32R):
    R2 = scal['R2']
    gp = scal['gp']
    R = scal['R']
    nc = bass.Bass("TRN2", target_bir_lowering=False, debug=False)

    chunk = nc.dram_tensor("chunk", [NB, TS, TS], F32, kind="ExternalInput")
    out_d = nc.dram_tensor("out", [NB, TS, TS, 2], F32, kind="ExternalOutput")
    cd = {}
    for name, shape in [("mcat", [TS, 2 * TS]), ("wcat", [TS, 2]),
                        ("trirex", [TS, TS]), ("trione", [TS, TS]),
                        ("rcol", [TS, 1]), ("onescol", [TS, 1]),
                        ("rprow", [1, TS]), ("onesrow", [1, TS]),
                        ("bsub", [TS, TS]), ("be127", [1, TS]),
                        ("ident", [TS, TS]),
                        ("vinrow", [1, NB]), ("xinrow", [1, NB])]:
        cd[name] = nc.dram_tensor(name, shape, F32, kind="ExternalInput")

    with tile.TileContext(nc) as tc:
        with (
            tc.tile_pool(name="const", bufs=1) as cpool,
            tc.tile_pool(name="pers", bufs=1) as pers,
            tc.tile_pool(name="yp", bufs=3) as ypool,
            tc.tile_pool(name="op", bufs=2) as opool,
            tc.tile_pool(name="tp", bufs=2, space="PSUM") as tpsum,
            tc.tile_pool(name="mp", bufs=3, space="PSUM") as mpsum,
            tc.tile_pool(name="sp", bufs=1, space="PSUM") as spsum,
            tc.tile_pool(name="lw", bufs=1, space="PSUM") as lwpsum,
        ):
            # ---- constants to SBUF
            cs = {}
            for name, t in cd.items():
                ct = cpool.tile(list(t.shape), F32, tag=f"c_{name}")
                nc.sync.dma_start(ct[:], t[:])
                cs[name] = ct

            # ---- persistent tiles
            xc = pers.tile([TS, NB, TS], F32, tag="xc")       # [j', b, p]
            phi = pers.tile([TS, NB], F32, tag="phi")
            lam = pers.tile([TS, NB], F32, tag="lam")
            w3s = pers.tile([TS, NB], F32, tag="w3s")
            eta = pers.tile([TS, NB], F32, tag="eta")
            cfin = pers.tile([TS, NB], F32, tag="cfin")
            xtm = pers.tile([TS, NB], F32, tag="xtm")
            delta = pers.tile([TS, NB], F32, tag="delta")
            cfinT = pers.tile([NB, TS], F32, tag="cfinT")
            brow = pers.tile([1, NB], F32, tag="brow")
            brow2 = pers.tile([1, NB], F32, tag="brow2")
            btmp = pers.tile([1, NB], F32, tag="btmp")

            lamw3 = lwpsum.tile([TS, TS], F32, tag="lamw3")   # cols 2b, 2b+1

            # ---- phase A: load, transpose to Xc, phase-1 matmuls
            for g in range(NGRP):
                yt = ypool.tile([TS, BPG, TS], F32, tag="yt")
                src = chunk[g * BPG:(g + 1) * BPG].rearrange("b p j -> p b j")
                nc.sync.dma_start(yt[:], src)
                # phi slice: j'=0 of each block
                nc.vector.tensor_copy(phi[:, g * BPG:(g + 1) * BPG],
                                      yt[:, :, 0])
                for bb in range(0, BPG, 2):
                    b = g * BPG + bb
                    tpt = tpsum.tile([TS, 2 * TS], F32, tag="tpt")
                    nc.tensor.transpose(tpt[:, 0:TS], yt[:, bb, :],
                                        cs["ident"][:])
                    nc.tensor.transpose(tpt[:, TS:2 * TS], yt[:, bb + 1, :],
                                        cs["ident"][:])
                    eng = nc.vector if (bb // 2) % 2 == 0 else nc.scalar
                    if eng is nc.vector:
                        eng.tensor_copy(xc[:, b:b + 2, :], tpt[:])
                    else:
                        eng.copy(xc[:, b:b + 2, :], tpt[:])
                    # phase-1: per-tile weighted sums (weights 0 at j'=0)
                    nc.tensor.matmul(lamw3[:, 2 * b:2 * b + 2],
                                     xc[:, b, :], cs["wcat"][:],
                                     start=True, stop=True)
                    nc.tensor.matmul(lamw3[:, 2 * (b + 1):2 * (b + 1) + 2],
                                     xc[:, b + 1, :], cs["wcat"][:],
                                     start=True, stop=True)

            # ---- phase B: tiny hierarchical scan for carries
            lw2 = lamw3[:].rearrange("p (b two) -> p b two", two=2)
            nc.vector.tensor_copy(lam[:], lw2[:, :, 0])
            nc.vector.tensor_copy(w3s[:], lw2[:, :, 1])

            etap = spsum.tile([TS, TS], F32, tag="spA")
            nc.tensor.matmul(etap[:, 0:NB], cs["bsub"][:], phi[:],
                             start=True, stop=False)
            nc.tensor.matmul(etap[:, 0:NB - 1], cs["be127"][:],
                             phi[0:1, 1:NB], start=False, stop=False)
            nc.tensor.matmul(etap[:, 0:NB], cs["ident"][:], lam[:],
                             start=False, stop=True)
            nc.vector.tensor_copy(eta[:], etap[:, 0:NB])

            clp = spsum.tile([TS, TS], F32, tag="spA")
            nc.tensor.matmul(clp[:, 0:NB], cs["trirex"][:], eta[:],
                             start=True, stop=False)
            blkp = spsum.tile([TS, TS], F32, tag="spB")
            nc.tensor.matmul(blkp[0:1, 0:NB], cs["rcol"][:], eta[:],
                             start=True, stop=True)
            nc.vector.tensor_copy(brow[:], blkp[0:1, 0:NB])

            # 64-wide inclusive scan with ratio R2, then shift to exclusive
            for s2 in (1, 2, 4, 8, 16, 32):
                nc.vector.tensor_scalar_mul(btmp[0:1, 0:NB - s2],
                                            brow[0:1, 0:NB - s2],
                                            float(R2 ** s2))
                nc.vector.tensor_add(brow[0:1, s2:NB], brow[0:1, s2:NB],
                                     btmp[0:1, 0:NB - s2])
            nc.vector.tensor_copy(btmp[0:1, 0:NB - 1], brow[0:1, 0:NB - 1])
            nc.vector.memset(brow[0:1, 0:1], 0.0)
            nc.vector.tensor_copy(brow[0:1, 1:NB], btmp[0:1, 0:NB - 1])
            nc.vector.tensor_add(brow[:], brow[:], cs["vinrow"][:])

            nc.tensor.matmul(clp[:, 0:NB], cs["rprow"][:], brow[0:1, :],
                             start=False, stop=True)
            nc.vector.tensor_copy(cfin[:], clp[:, 0:NB])

            # delta = dt*((gp-1+R)*cfin + w3s + eta)
            k1 = float(DT * (gp - 1.0 + R))
            nc.vector.tensor_scalar_mul(delta[:], cfin[:], k1)
            nc.vector.tensor_add(lam[:], w3s[:], eta[:])   # lam reused as tmp
            nc.vector.tensor_scalar_mul(lam[:], lam[:], float(DT))
            nc.vector.tensor_add(delta[:], delta[:], lam[:])

            xlp = spsum.tile([TS, TS], F32, tag="spA")
            nc.tensor.matmul(xlp[:, 0:NB], cs["trione"][:], delta[:],
                             start=True, stop=False)
            xbp = spsum.tile([TS, TS], F32, tag="spB")
            nc.tensor.matmul(xbp[0:1, 0:NB], cs["onescol"][:], delta[:],
                             start=True, stop=True)
            nc.vector.tensor_copy(brow2[:], xbp[0:1, 0:NB])
            for s2 in (1, 2, 4, 8, 16, 32):
                nc.vector.tensor_copy(btmp[0:1, 0:NB - s2],
                                      brow2[0:1, 0:NB - s2])
                nc.vector.tensor_add(brow2[0:1, s2:NB], brow2[0:1, s2:NB],
                                     btmp[0:1, 0:NB - s2])
            nc.vector.tensor_copy(btmp[0:1, 0:NB - 1], brow2[0:1, 0:NB - 1])
            nc.vector.memset(brow2[0:1, 0:1], 0.0)
            nc.vector.tensor_copy(brow2[0:1, 1:NB], btmp[0:1, 0:NB - 1])
            nc.vector.tensor_add(brow2[:], brow2[:], cs["xinrow"][:])

            nc.tensor.matmul(xlp[:, 0:NB], cs["onesrow"][:], brow2[0:1, :],
                             start=False, stop=True)
            nc.vector.tensor_copy(xtm[:], xlp[:, 0:NB])

            # carries -> Xc row 0 (via PE transpose + SBUF->SBUF DMA)
            ctp = spsum.tile([TS, TS], F32, tag="spA")
            nc.tensor.transpose(ctp[0:NB, :], cfin[:], cs["ident"][:])
            nc.vector.tensor_copy(cfinT[:], ctp[0:NB, :])
            nc.sync.dma_start(xc[0:1, :, :], cfinT[:])

            # ---- phase C: main matmuls + interleaved output
            for g in range(NGRP):
                ot = opool.tile([TS, BPG, TS, 2], F32, tag="ot")
                for bb in range(0, BPG, 2):
                    b = g * BPG + bb
                    mpt = mpsum.tile([TS, 4 * TS], F32, tag="mpt")
                    for k in range(2):
                        lhs = xc[:, b + k, :]
                        rhs = cs["mcat"][:]
                        if use_f32r:
                            lhs = lhs.bitcast(mybir.dt.float32r)
                            rhs = rhs.bitcast(mybir.dt.float32r)
                        nc.tensor.matmul(mpt[:, 2 * TS * k:2 * TS * (k + 1)],
                                         lhs, rhs, start=True, stop=True)
                    # v (cols 0:128 of each half) -> interleaved slot 1
                    mview = mpt[:].rearrange("p (k h i) -> p k h i",
                                             k=2, h=2)
                    nc.vector.tensor_copy(ot[:, bb:bb + 2, :, 1],
                                          mview[:, :, 0, :])
                    # x (cols 128:256 of each half) + per-tile base -> slot 0
                    nc.scalar.add(ot[:, bb, :, 0], mview[:, 0, 1, :],
                                  xtm[:, b:b + 1])
                    nc.scalar.add(ot[:, bb + 1, :, 0], mview[:, 1, 1, :],
                                  xtm[:, b + 1:b + 2])
                dst = out_d[g * BPG:(g + 1) * BPG].rearrange(
                    "b p i c -> p b i c")
                nc.sync.dma_start(dst, ot[:])

    _split_multiwaits(nc)
    return nc


# ---------------------------------------------------------------- entry point

_NC_CACHE = {}
LAST_RESULTS = None


def kernel(initial_state, actions, mass, friction_coeff):
    initial_state = np.asarray(initial_state, np.float32)
    actions = np.asarray(actions, np.float32)
    m_safe = abs(float(mass)) + 0.001
    A = 1.0 - float(friction_coeff) * DT / m_safe
    B = DT / m_safe

    F64 = actions.astype(np.float64).ravel()
    v0 = float(initial_state[0, 1])
    x0 = float(initial_state[0, 0])
    v_in, x_in = _host_carries(F64, v0, x0, A, B)
    cst, scal = _device_consts(A, B)

    key = (round(A, 15), round(B, 15))
    if key not in _NC_CACHE:
        _NC_CACHE[key] = _build_nc(scal)
    nc = _NC_CACHE[key]

    Fpad = np.empty(T, np.float32)
    Fpad[0] = 0.0
    Fpad[1:] = actions.ravel()[:-1]

    R2 = scal['R2']
    in_maps = []
    for d in range(NCORES):
        m = {"chunk": np.ascontiguousarray(
                Fpad[d * C:(d + 1) * C].reshape(NB, TS, TS))}
        m.update({k: v for k, v in cst.items()})
        m["vinrow"] = np.ascontiguousarray(
            (v_in[d] * R2 ** np.arange(NB))[None, :], np.float32)
        m["xinrow"] = np.full((1, NB), x_in[d], np.float32)
        in_maps.append(m)

    global LAST_RESULTS
    res = run_bass_kernel_spmd(nc, in_maps, core_ids=list(range(NCORES)))
    LAST_RESULTS = res

    out = np.empty((T + 1, 2), np.float32)
    for d in range(NCORES):
        out[d * C:(d + 1) * C] = res.results[d]["out"].reshape(C, 2)
    out[T, 0] = x_in[NCORES]
    out[T, 1] = v_in[NCORES]
    return out


if __name__ == "__main__":
    rng = np.random.default_rng(0)
    ins = {
        "initial_state": rng.standard_normal((1, 2)).astype(np.float32),
        "actions": rng.standard_normal((T, 1)).astype(np.float32),
        "mass": np.float32(5.0),
        "friction_coeff": np.float32(0.5),
    }
    got = kernel(**ins)
    # float64 sequential check via scipy
    from scipy.signal import lfilter
    m_safe = abs(float(ins["mass"])) + 0.001
    A = 1.0 - float(ins["friction_coeff"]) * DT / m_safe
    B = DT / m_safe
    F = ins["actions"][:, 0].astype(np.float64)
    v, _ = lfilter([B], [1, -A], F, zi=np.array([A * float(ins["initial_state"][0, 1])]))
    x = float(ins["initial_state"][0, 0]) + DT * np.cumsum(v)
    exp = np.empty((T + 1, 2))
    exp[0] = ins["initial_state"][0]
    exp[1:, 0] = x
    exp[1:, 1] = v
    rel = np.linalg.norm(got - exp) / np.linalg.norm(exp)
    print("rel err (fro) vs float64 sequential:", rel)
    print("max abs err:", np.abs(got - exp).max())


# revision 7
# speedup vs baseline: 1.0141x; 1.0141x over previous
"""Trainium2 Bass kernel for nn_DifferentiableSimulator.

Math: Euler integration of 1D kinematics is a constant-coefficient linear
recurrence  v' = A*v + B*F  (A = 1 - c*dt/m_safe, B = dt/m_safe) followed by
a cumulative sum  x' = x + dt*v'.

Sequence-parallel strategy over 8 cores (T split into 8 chunks of C=1M):
 - Host (float64): the 8 chunk-boundary states (v_in, x_in per core) via
   geometric-weight dot products — O(T) vectorized numpy, exact enough.
 - Device (per core): blocked associative scan. The chunk is cut into 8192
   tiles of 128 steps. Each tile's entire scan is ONE 128x128->[128,256]
   matmul: lhsT = Xc_block (row 0 = the tile's incoming state c_k, rows
   1..127 = forces), rhs = [Mv | Mx] constant weight matrices. Tile carries
   c_k come from a cheap hierarchical scan (per-tile weighted sums via tiny
   matmuls, 128-wide column scan via triangular matmul, 64-block row scan
   via log-step), and per-tile x offsets are added for free as the
   per-partition bias of the ScalarE activation that drains PSUM.
Everything ends in an interleaved [x, v] SBUF layout that DMAs out with 1KB
contiguous bursts.
"""
import numpy as np
import concourse.bass as bass
import concourse.mybir as mybir
import concourse.tile as tile
from concourse.bass_utils import run_bass_kernel_spmd

DT = 0.01
T = 8_388_608
NCORES = 8
C = T // NCORES          # 1,048,576 timesteps per core
NB = 64                  # blocks per core (each block = 128 tiles = 16384 steps)
TS = 128                 # tile size (steps per tile) == partitions
NGRP = 8                 # DMA pipeline groups
BPG = NB // NGRP         # blocks per group
F32 = mybir.dt.float32

USE_F32R = False          # main matmuls in float32r (4x faster PE, validated)


# ---------------------------------------------------------------- host math

def _host_carries(F64, v0, x0, A, B):
    v_in = np.zeros(NCORES + 1)
    x_in = np.zeros(NCORES + 1)
    v_in[0] = v0
    x_in[0] = x0
    j = np.arange(C, dtype=np.float64)
    w_v = A ** (C - 1 - j)
    if abs(1.0 - A) > 1e-12:
        S = (1.0 - A ** (C - j)) / (1.0 - A)
        G = DT * A * (1.0 - A ** C) / (1.0 - A)
    else:
        S = C - j
        G = DT * C
    RC = A ** C
    for d in range(NCORES):
        Fc = F64[d * C:(d + 1) * C]
        x_in[d + 1] = x_in[d] + v_in[d] * G + DT * B * np.dot(S, Fc)
        v_in[d + 1] = RC * v_in[d] + B * np.dot(w_v, Fc)
    return v_in, x_in


def _device_consts(A, B):
    """Constant matrices (float64 host math, shipped as fp32)."""
    i = np.arange(TS)
    s = np.arange(TS - 1)[:, None]
    e = i[None, :] - 1 - s
    Mv = np.zeros((TS, TS))
    Mv[0, :] = A ** i
    Mv[1:, :] = np.where(e >= 0, B * A ** np.maximum(e, 0), 0.0)
    ap = A ** np.arange(TS + 1)
    cum = np.cumsum(ap)
    Mx = np.zeros((TS, TS))
    Mx[0, :] = DT * (cum[i] - 1.0)
    Mx[1:, :] = np.where(e >= 0, DT * B * cum[np.maximum(e, 0)], 0.0)
    jj = np.arange(TS)
    wlam = np.where(jj >= 1, B * A ** (128 - jj), 0.0)
    w3w = np.where(jj >= 1, B * cum[127 - jj], 0.0)
    wcat = np.stack([wlam, w3w], axis=1)
    R = A ** TS
    q = np.arange(TS)[:, None]
    m = np.arange(TS)[None, :]
    TriRex = np.where(q <= m - 1, R ** np.maximum(m - 1 - q, 0), 0.0)
    TriOnesEx = np.where(q <= m - 1, 1.0, 0.0)
    Rcol = (R ** (127 - np.arange(TS)))[:, None]
    OnesCol = np.ones((TS, 1))
    Rprow = (R ** np.arange(TS))[None, :]
    OnesRow = np.ones((1, TS))
    Bsub = np.zeros((TS, TS))
    Bsub[np.arange(1, TS), np.arange(TS - 1)] = B
    Be127 = np.zeros((1, TS))
    Be127[0, 127] = B
    Ident = np.eye(TS)
    cst = dict(mcat=np.concatenate([Mv, Mx], axis=1), wcat=wcat,
               trirex=TriRex, trione=TriOnesEx, rcol=Rcol, onescol=OnesCol,
               rprow=Rprow, onesrow=OnesRow, bsub=Bsub, be127=Be127,
               ident=Ident)
    cst = {k: np.ascontiguousarray(v, np.float32) for k, v in cst.items()}
    scal = dict(R=R, R2=R ** TS, gp=float(cum[127]))
    return cst, scal


# ---------------------------------------------------------------- bass build

def _split_multiwaits(nc, maxw=1):
    """The walrus build in this container accepts at most ONE sync wait per
    instruction (Tile's scheduler happily attaches several). Keep the first
    wait on the instruction; move extras onto EventSemaphore instructions
    inserted immediately before, same engine."""
    n = 0
    for fn in nc.m.functions:
        for bb in fn.blocks:
            new_insts = []
            changed = False
            for inst in bb.instructions:
                si = inst.sync_info
                waits = list(si.on_wait) if si and si.on_wait else []
                if len(waits) > maxw:
                    changed = True
                    for w in waits[maxw:]:
                        ev = mybir.InstEventSemaphore(
                            name=f"{inst.name}-ws{n}", engine=inst.engine,
                            ins=[], outs=[],
                            sync_info=mybir.SyncInfo(on_wait=[w], on_update=[]),
                        )
                        new_insts.append(ev)
                        n += 1
                    si.on_wait = waits[:maxw]
                new_insts.append(inst)
            if changed:
                bb.instructions = new_insts
    return n


def _build_nc(scal, use_f32r=USE_F32R):
    R2 = scal['R2']
    gp = scal['gp']
    R = scal['R']
    nc = bass.Bass("TRN2", target_bir_lowering=False, debug=False)

    chunk = nc.dram_tensor("chunk", [NB, TS, TS], F32, kind="ExternalInput")
    out_d = nc.dram_tensor("out", [NB, TS, TS, 2], F32, kind="ExternalOutput")
    cd = {}
    for name, shape in [("mcat", [TS, 2 * TS]), ("wcat", [TS, 2]),
                        ("trirex", [TS, TS]), ("trione", [TS, TS]),
                        ("rcol", [TS, 1]), ("onescol", [TS, 1]),
                        ("rprow", [1, TS]), ("onesrow", [1, TS]),
                        ("bsub", [TS, TS]), ("be127", [1, TS]),
                        ("ident", [TS, TS]),
                        ("vinrow", [1, NB]), ("xinrow", [1, NB])]:
        cd[name] = nc.dram_tensor(name, shape, F32, kind="ExternalInput")

    with tile.TileContext(nc) as tc:
        with (
            tc.tile_pool(name="const", bufs=1) as cpool,
            tc.tile_pool(name="pers", bufs=1) as pers,
            tc.tile_pool(name="yp", bufs=3) as ypool,
            tc.tile_pool(name="op", bufs=2) as opool,
            tc.tile_pool(name="tp", bufs=2, space="PSUM") as tpsum,
            tc.tile_pool(name="mp", bufs=3, space="PSUM") as mpsum,
            tc.tile_pool(name="sp", bufs=1, space="PSUM") as spsum,
            tc.tile_pool(name="lw", bufs=1, space="PSUM") as lwpsum,
        ):
            # ---- constants to SBUF
            cs = {}
            for name, t in cd.items():
                ct = cpool.tile(list(t.shape), F32, tag=f"c_{name}")
                nc.sync.dma_start(ct[:], t[:])
                cs[name] = ct

            # ---- persistent tiles
            xc = pers.tile([TS, NB, TS], F32, tag="xc")       # [j', b, p]
            phi = pers.tile([TS, NB], F32, tag="phi")
            lam = pers.tile([TS, NB], F32, tag="lam")
            w3s = pers.tile([TS, NB], F32, tag="w3s")
            eta = pers.tile([TS, NB], F32, tag="eta")
            cfin = pers.tile([TS, NB], F32, tag="cfin")
            xtm = pers.tile([TS, NB], F32, tag="xtm")
            delta = pers.tile([TS, NB], F32, tag="delta")
            cfinT = pers.tile([NB, TS], F32, tag="cfinT")
            brow = pers.tile([1, NB], F32, tag="brow")
            brow2 = pers.tile([1, NB], F32, tag="brow2")
            btmp = pers.tile([1, NB], F32, tag="btmp")

            lamw3 = lwpsum.tile([TS, TS], F32, tag="lamw3")   # cols 2b, 2b+1

            # ---- phase A: load, transpose to Xc (dense on PE), then
            # phase-1 matmuls (dense, after copies land)
            for g in range(NGRP):
                yt = ypool.tile([TS, BPG, TS], F32, tag="yt")
                src = chunk[g * BPG:(g + 1) * BPG].rearrange("b p j -> p b j")
                nc.sync.dma_start(yt[:], src)
                # phi slice: j'=0 of each block
                nc.vector.tensor_copy(phi[:, g * BPG:(g + 1) * BPG],
                                      yt[:, :, 0])
                for bb in range(0, BPG, 2):
                    b = g * BPG + bb
                    tpt = tpsum.tile([TS, 2 * TS], F32, tag="tpt")
                    nc.tensor.transpose(tpt[:, 0:TS], yt[:, bb, :],
                                        cs["ident"][:])
                    nc.tensor.transpose(tpt[:, TS:2 * TS], yt[:, bb + 1, :],
                                        cs["ident"][:])
                    eng = nc.vector if (bb // 2) % 2 == 0 else nc.scalar
                    if eng is nc.vector:
                        eng.tensor_copy(xc[:, b:b + 2, :], tpt[:])
                    else:
                        eng.copy(xc[:, b:b + 2, :], tpt[:])
            for b in range(NB):
                # phase-1: per-tile weighted sums (weights 0 at j'=0)
                nc.tensor.matmul(lamw3[:, 2 * b:2 * b + 2],
                                 xc[:, b, :], cs["wcat"][:],
                                 start=True, stop=True)

            # ---- phase B: tiny hierarchical scan for carries
            lw2 = lamw3[:].rearrange("p (b two) -> p b two", two=2)
            nc.vector.tensor_copy(lam[:], lw2[:, :, 0])
            nc.vector.tensor_copy(w3s[:], lw2[:, :, 1])

            etap = spsum.tile([TS, TS], F32, tag="spA")
            nc.tensor.matmul(etap[:, 0:NB], cs["bsub"][:], phi[:],
                             start=True, stop=False)
            nc.tensor.matmul(etap[:, 0:NB - 1], cs["be127"][:],
                             phi[0:1, 1:NB], start=False, stop=False)
            nc.tensor.matmul(etap[:, 0:NB], cs["ident"][:], lam[:],
                             start=False, stop=True)
            nc.vector.tensor_copy(eta[:], etap[:, 0:NB])

            clp = spsum.tile([TS, TS], F32, tag="spA")
            nc.tensor.matmul(clp[:, 0:NB], cs["trirex"][:], eta[:],
                             start=True, stop=False)
            blkp = spsum.tile([TS, TS], F32, tag="spB")
            nc.tensor.matmul(blkp[0:1, 0:NB], cs["rcol"][:], eta[:],
                             start=True, stop=True)
            nc.vector.tensor_copy(brow[:], blkp[0:1, 0:NB])

            # 64-wide inclusive scan with ratio R2, then shift to exclusive
            for s2 in (1, 2, 4, 8, 16, 32):
                nc.vector.tensor_scalar_mul(btmp[0:1, 0:NB - s2],
                                            brow[0:1, 0:NB - s2],
                                            float(R2 ** s2))
                nc.vector.tensor_add(brow[0:1, s2:NB], brow[0:1, s2:NB],
                                     btmp[0:1, 0:NB - s2])
            nc.vector.tensor_copy(btmp[0:1, 0:NB - 1], brow[0:1, 0:NB - 1])
            nc.vector.memset(brow[0:1, 0:1], 0.0)
            nc.vector.tensor_copy(brow[0:1, 1:NB], btmp[0:1, 0:NB - 1])
            nc.vector.tensor_add(brow[:], brow[:], cs["vinrow"][:])

            nc.tensor.matmul(clp[:, 0:NB], cs["rprow"][:], brow[0:1, :],
                             start=False, stop=True)
            nc.vector.tensor_copy(cfin[:], clp[:, 0:NB])

            # delta = dt*((gp-1+R)*cfin + w3s + eta)
            k1 = float(DT * (gp - 1.0 + R))
            nc.vector.tensor_scalar_mul(delta[:], cfin[:], k1)
            nc.vector.tensor_add(lam[:], w3s[:], eta[:])   # lam reused as tmp
            nc.vector.tensor_scalar_mul(lam[:], lam[:], float(DT))
            nc.vector.tensor_add(delta[:], delta[:], lam[:])

            xlp = spsum.tile([TS, TS], F32, tag="spA")
            nc.tensor.matmul(xlp[:, 0:NB], cs["trione"][:], delta[:],
                             start=True, stop=False)
            xbp = spsum.tile([TS, TS], F32, tag="spB")
            nc.tensor.matmul(xbp[0:1, 0:NB], cs["onescol"][:], delta[:],
                             start=True, stop=True)
            nc.vector.tensor_copy(brow2[:], xbp[0:1, 0:NB])
            for s2 in (1, 2, 4, 8, 16, 32):
                nc.vector.tensor_copy(btmp[0:1, 0:NB - s2],
                                      brow2[0:1, 0:NB - s2])
                nc.vector.tensor_add(brow2[0:1, s2:NB], brow2[0:1, s2:NB],
                                     btmp[0:1, 0:NB - s2])
            nc.vector.tensor_copy(btmp[0:1, 0:NB - 1], brow2[0:1, 0:NB - 1])
            nc.vector.memset(brow2[0:1, 0:1], 0.0)
            nc.vector.tensor_copy(brow2[0:1, 1:NB], btmp[0:1, 0:NB - 1])
            nc.vector.tensor_add(brow2[:], brow2[:], cs["xinrow"][:])

            nc.tensor.matmul(xlp[:, 0:NB], cs["onesrow"][:], brow2[0:1, :],
                             start=False, stop=True)
            nc.vector.tensor_copy(xtm[:], xlp[:, 0:NB])

            # carries -> Xc row 0 (via PE transpose + SBUF->SBUF DMA)
            ctp = spsum.tile([TS, TS], F32, tag="spA")
            nc.tensor.transpose(ctp[0:NB, :], cfin[:], cs["ident"][:])
            nc.vector.tensor_copy(cfinT[:], ctp[0:NB, :])
            nc.sync.dma_start(xc[0:1, :, :], cfinT[:])

            # ---- phase C: main matmuls + interleaved output
            for g in range(NGRP):
                ot = opool.tile([TS, BPG, TS, 2], F32, tag="ot")
                for bb in range(0, BPG, 2):
                    b = g * BPG + bb
                    mpt = mpsum.tile([TS, 4 * TS], F32, tag="mpt")
                    for k in range(2):
                        lhs = xc[:, b + k, :]
                        rhs = cs["mcat"][:]
                        if use_f32r:
                            lhs = lhs.bitcast(mybir.dt.float32r)
                            rhs = rhs.bitcast(mybir.dt.float32r)
                        nc.tensor.matmul(mpt[:, 2 * TS * k:2 * TS * (k + 1)],
                                         lhs, rhs, start=True, stop=True)
                    # v (cols 0:128 of each half) -> interleaved slot 1
                    mview = mpt[:].rearrange("p (k h i) -> p k h i",
                                             k=2, h=2)
                    nc.vector.tensor_copy(ot[:, bb:bb + 2, :, 1],
                                          mview[:, :, 0, :])
                    # x (cols 128:256 of each half) + per-tile base -> slot 0
                    nc.scalar.add(ot[:, bb, :, 0], mview[:, 0, 1, :],
                                  xtm[:, b:b + 1])
                    nc.scalar.add(ot[:, bb + 1, :, 0], mview[:, 1, 1, :],
                                  xtm[:, b + 1:b + 2])
                dst = out_d[g * BPG:(g + 1) * BPG].rearrange(
                    "b p i c -> p b i c")
                nc.scalar.dma_start(dst, ot[:])

    _split_multiwaits(nc)
    return nc


# ---------------------------------------------------------------- entry point

_NC_CACHE = {}
LAST_RESULTS = None


def kernel(initial_state, actions, mass, friction_coeff):
    initial_state = np.asarray(initial_state, np.float32)
    actions = np.asarray(actions, np.float32)
    m_safe = abs(float(mass)) + 0.001
    A = 1.0 - float(friction_coeff) * DT / m_safe
    B = DT / m_safe

    F64 = actions.astype(np.float64).ravel()
    v0 = float(initial_state[0, 1])
    x0 = float(initial_state[0, 0])
    v_in, x_in = _host_carries(F64, v0, x0, A, B)
    cst, scal = _device_consts(A, B)

    key = (round(A, 15), round(B, 15))
    if key not in _NC_CACHE:
        _NC_CACHE[key] = _build_nc(scal)
    nc = _NC_CACHE[key]

    Fpad = np.empty(T, np.float32)
    Fpad[0] = 0.0
    Fpad[1:] = actions.ravel()[:-1]

    R2 = scal['R2']
    in_maps = []
    for d in range(NCORES):
        m = {"chunk": np.ascontiguousarray(
                Fpad[d * C:(d + 1) * C].reshape(NB, TS, TS))}
        m.update({k: v for k, v in cst.items()})
        m["vinrow"] = np.ascontiguousarray(
            (v_in[d] * R2 ** np.arange(NB))[None, :], np.float32)
        m["xinrow"] = np.full((1, NB), x_in[d], np.float32)
        in_maps.append(m)

    global LAST_RESULTS
    res = run_bass_kernel_spmd(nc, in_maps, core_ids=list(range(NCORES)))
    LAST_RESULTS = res

    out = np.empty((T + 1, 2), np.float32)
    for d in range(NCORES):
        out[d * C:(d + 1) * C] = res.results[d]["out"].reshape(C, 2)
    out[T, 0] = x_in[NCORES]
    out[T, 1] = v_in[NCORES]
    return out


if __name__ == "__main__":
    rng = np.random.default_rng(0)
    ins = {
        "initial_state": rng.standard_normal((1, 2)).astype(np.float32),
        "actions": rng.standard_normal((T, 1)).astype(np.float32),
        "mass": np.float32(5.0),
        "friction_coeff": np.float32(0.5),
    }
    got = kernel(**ins)
    # float64 sequential check via scipy
    from scipy.signal import lfilter
    m_safe = abs(float(ins["mass"])) + 0.001
    A = 1.0 - float(ins["friction_coeff"]) * DT / m_safe
    B = DT / m_safe
    F = ins["actions"][:, 0].astype(np.float64)
    v, _ = lfilter([B], [1, -A], F, zi=np.array([A * float(ins["initial_state"][0, 1])]))
    x = float(ins["initial_state"][0, 0]) + DT * np.cumsum(v)
    exp = np.empty((T + 1, 2))
    exp[0] = ins["initial_state"][0]
    exp[1:, 0] = x
    exp[1:, 1] = v
    rel = np.linalg.norm(got - exp) / np.linalg.norm(exp)
    print("rel err (fro) vs float64 sequential:", rel)
    print("max abs err:", np.abs(got - exp).max())


# revision 8
# speedup vs baseline: 1.0627x; 1.0479x over previous
"""Trainium2 Bass kernel for nn_DifferentiableSimulator.

Math: Euler integration of 1D kinematics is a constant-coefficient linear
recurrence  v' = A*v + B*F  (A = 1 - c*dt/m_safe, B = dt/m_safe) followed by
a cumulative sum  x' = x + dt*v'.

Sequence-parallel strategy over 8 cores (T split into 8 chunks of C=1M):
 - Host (float64): the 8 chunk-boundary states (v_in, x_in per core) via
   geometric-weight dot products — O(T) vectorized numpy, exact enough.
 - Device (per core): blocked associative scan. The chunk is cut into 8192
   tiles of 128 steps. Each tile's entire scan is ONE 128x128->[128,256]
   matmul: lhsT = Xc_block (row 0 = the tile's incoming state c_k, rows
   1..127 = forces), rhs = [Mv | Mx] constant weight matrices. Tile carries
   c_k come from a cheap hierarchical scan (per-tile weighted sums via tiny
   matmuls, 128-wide column scan via triangular matmul, 64-block row scan
   via log-step), and per-tile x offsets are added for free as the
   per-partition bias of the ScalarE activation that drains PSUM.
Everything ends in an interleaved [x, v] SBUF layout that DMAs out with 1KB
contiguous bursts.
"""
import numpy as np
import concourse.bass as bass
import concourse.mybir as mybir
import concourse.tile as tile
from concourse.bass_utils import run_bass_kernel_spmd

DT = 0.01
T = 8_388_608
NCORES = 8
C = T // NCORES          # 1,048,576 timesteps per core
NB = 64                  # blocks per core (each block = 128 tiles = 16384 steps)
TS = 128                 # tile size (steps per tile) == partitions
NGRP = 8                 # DMA pipeline groups
BPG = NB // NGRP         # blocks per group
F32 = mybir.dt.float32
F32R = mybir.dt.float32r

USE_F32R = True          # main matmuls in float32r (4x faster PE, validated)


# ---------------------------------------------------------------- host math

def _host_carries(F64, v0, x0, A, B):
    v_in = np.zeros(NCORES + 1)
    x_in = np.zeros(NCORES + 1)
    v_in[0] = v0
    x_in[0] = x0
    j = np.arange(C, dtype=np.float64)
    w_v = A ** (C - 1 - j)
    if abs(1.0 - A) > 1e-12:
        S = (1.0 - A ** (C - j)) / (1.0 - A)
        G = DT * A * (1.0 - A ** C) / (1.0 - A)
    else:
        S = C - j
        G = DT * C
    RC = A ** C
    for d in range(NCORES):
        Fc = F64[d * C:(d + 1) * C]
        x_in[d + 1] = x_in[d] + v_in[d] * G + DT * B * np.dot(S, Fc)
        v_in[d + 1] = RC * v_in[d] + B * np.dot(w_v, Fc)
    return v_in, x_in


def _device_consts(A, B):
    """Constant matrices (float64 host math, shipped as fp32)."""
    i = np.arange(TS)
    s = np.arange(TS - 1)[:, None]
    e = i[None, :] - 1 - s
    Mv = np.zeros((TS, TS))
    Mv[0, :] = A ** i
    Mv[1:, :] = np.where(e >= 0, B * A ** np.maximum(e, 0), 0.0)
    ap = A ** np.arange(TS + 1)
    cum = np.cumsum(ap)
    Mx = np.zeros((TS, TS))
    Mx[0, :] = DT * (cum[i] - 1.0)
    Mx[1:, :] = np.where(e >= 0, DT * B * cum[np.maximum(e, 0)], 0.0)
    jj = np.arange(TS)
    wlam = np.where(jj >= 1, B * A ** (128 - jj), 0.0)
    w3w = np.where(jj >= 1, B * cum[127 - jj], 0.0)
    wcat = np.stack([wlam, w3w], axis=1)
    R = A ** TS
    q = np.arange(TS)[:, None]
    m = np.arange(TS)[None, :]
    TriRex = np.where(q <= m - 1, R ** np.maximum(m - 1 - q, 0), 0.0)
    TriOnesEx = np.where(q <= m - 1, 1.0, 0.0)
    Rcol = (R ** (127 - np.arange(TS)))[:, None]
    OnesCol = np.ones((TS, 1))
    Rprow = (R ** np.arange(TS))[None, :]
    OnesRow = np.ones((1, TS))
    Bsub = np.zeros((TS, TS))
    Bsub[np.arange(1, TS), np.arange(TS - 1)] = B
    Be127 = np.zeros((1, TS))
    Be127[0, 127] = B
    Ident = np.eye(TS)
    cst = dict(mcat=np.concatenate([Mv, Mx], axis=1), wcat=wcat,
               trirex=TriRex, trione=TriOnesEx, rcol=Rcol, onescol=OnesCol,
               rprow=Rprow, onesrow=OnesRow, bsub=Bsub, be127=Be127,
               ident=Ident)
    cst = {k: np.ascontiguousarray(v, np.float32) for k, v in cst.items()}
    scal = dict(R=R, R2=R ** TS, gp=float(cum[127]))
    return cst, scal


# ---------------------------------------------------------------- bass build

def _split_multiwaits(nc, maxw=1):
    """The walrus build in this container accepts at most ONE sync wait per
    instruction (Tile's scheduler happily attaches several). Keep the first
    wait on the instruction; move extras onto EventSemaphore instructions
    inserted immediately before, same engine."""
    n = 0
    for fn in nc.m.functions:
        for bb in fn.blocks:
            new_insts = []
            changed = False
            for inst in bb.instructions:
                si = inst.sync_info
                waits = list(si.on_wait) if si and si.on_wait else []
                if len(waits) > maxw:
                    changed = True
                    for w in waits[maxw:]:
                        ev = mybir.InstEventSemaphore(
                            name=f"{inst.name}-ws{n}", engine=inst.engine,
                            ins=[], outs=[],
                            sync_info=mybir.SyncInfo(on_wait=[w], on_update=[]),
                        )
                        new_insts.append(ev)
                        n += 1
                    si.on_wait = waits[:maxw]
                new_insts.append(inst)
            if changed:
                bb.instructions = new_insts
    return n


def _build_nc(scal, use_f32r=USE_F32R):
    R2 = scal['R2']
    gp = scal['gp']
    R = scal['R']
    nc = bass.Bass("TRN2", target_bir_lowering=False, debug=False)

    chunk = nc.dram_tensor("chunk", [NB, TS, TS], F32, kind="ExternalInput")
    out_d = nc.dram_tensor("out", [NB, TS, TS, 2], F32, kind="ExternalOutput")
    cd = {}
    for name, shape in [("mcat", [TS, 2 * TS]), ("wcat", [TS, 2]),
                        ("trirex", [TS, TS]), ("trione", [TS, TS]),
                        ("rcol", [TS, 1]), ("onescol", [TS, 1]),
                        ("rprow", [1, TS]), ("onesrow", [1, TS]),
                        ("bsub", [TS, TS]), ("be127", [1, TS]),
                        ("ident", [TS, TS]),
                        ("vinrow", [1, NB]), ("xinrow", [1, NB])]:
        cd[name] = nc.dram_tensor(name, shape, F32, kind="ExternalInput")

    with tile.TileContext(nc) as tc:
        with (
            tc.tile_pool(name="const", bufs=1) as cpool,
            tc.tile_pool(name="pers", bufs=1) as pers,
            tc.tile_pool(name="yp", bufs=3) as ypool,
            tc.tile_pool(name="op", bufs=2) as opool,
            tc.tile_pool(name="tp", bufs=2, space="PSUM") as tpsum,
            tc.tile_pool(name="mp", bufs=3, space="PSUM") as mpsum,
            tc.tile_pool(name="sp", bufs=1, space="PSUM") as spsum,
            tc.tile_pool(name="lw", bufs=1, space="PSUM") as lwpsum,
        ):
            # ---- constants to SBUF
            cs = {}
            for name, t in cd.items():
                ct = cpool.tile(list(t.shape), F32, tag=f"c_{name}")
                nc.sync.dma_start(ct[:], t[:])
                cs[name] = ct
            if use_f32r:
                # rounded copies for fp32r matmul operands
                for name in ("mcat", "wcat"):
                    rt = cpool.tile(list(cd[name].shape), F32R,
                                    tag=f"r_{name}")
                    nc.vector.tensor_copy(rt[:], cs[name][:])
                    cs[name + "_r"] = rt

            # ---- persistent tiles
            xc = pers.tile([TS, NB, TS], F32R if use_f32r else F32, tag="xc")       # [j', b, p]
            phi = pers.tile([TS, NB], F32, tag="phi")
            lam = pers.tile([TS, NB], F32, tag="lam")
            w3s = pers.tile([TS, NB], F32, tag="w3s")
            eta = pers.tile([TS, NB], F32, tag="eta")
            cfin = pers.tile([TS, NB], F32, tag="cfin")
            xtm = pers.tile([TS, NB], F32, tag="xtm")
            delta = pers.tile([TS, NB], F32, tag="delta")
            cfinT = pers.tile([NB, TS], F32R if use_f32r else F32, tag="cfinT")
            brow = pers.tile([1, NB], F32, tag="brow")
            brow2 = pers.tile([1, NB], F32, tag="brow2")
            btmp = pers.tile([1, NB], F32, tag="btmp")

            lamw3 = lwpsum.tile([TS, TS], F32, tag="lamw3")   # cols 2b, 2b+1

            # ---- phase A: load, transpose to Xc (dense on PE), then
            # phase-1 matmuls (dense, after copies land)
            for g in range(NGRP):
                yt = ypool.tile([TS, BPG, TS], F32, tag="yt")
                src = chunk[g * BPG:(g + 1) * BPG].rearrange("b p j -> p b j")
                nc.sync.dma_start(yt[:], src)
                # phi slice: j'=0 of each block
                nc.vector.tensor_copy(phi[:, g * BPG:(g + 1) * BPG],
                                      yt[:, :, 0])
                for bb in range(0, BPG, 2):
                    b = g * BPG + bb
                    tpt = tpsum.tile([TS, 2 * TS], F32, tag="tpt")
                    nc.tensor.transpose(tpt[:, 0:TS], yt[:, bb, :],
                                        cs["ident"][:])
                    nc.tensor.transpose(tpt[:, TS:2 * TS], yt[:, bb + 1, :],
                                        cs["ident"][:])
                    eng = nc.vector if (bb // 2) % 2 == 0 else nc.scalar
                    if eng is nc.vector:
                        eng.tensor_copy(xc[:, b:b + 2, :], tpt[:])
                    else:
                        eng.copy(xc[:, b:b + 2, :], tpt[:])
            wcat_mm = cs["wcat_r"] if use_f32r else cs["wcat"]
            for b in range(NB):
                # phase-1: per-tile weighted sums (weights 0 at j'=0)
                nc.tensor.matmul(lamw3[:, 2 * b:2 * b + 2],
                                 xc[:, b, :], wcat_mm[:],
                                 start=True, stop=True)

            # ---- phase B: tiny hierarchical scan for carries
            lw2 = lamw3[:].rearrange("p (b two) -> p b two", two=2)
            nc.vector.tensor_copy(lam[:], lw2[:, :, 0])
            nc.vector.tensor_copy(w3s[:], lw2[:, :, 1])

            etap = spsum.tile([TS, TS], F32, tag="spA")
            nc.tensor.matmul(etap[:, 0:NB], cs["bsub"][:], phi[:],
                             start=True, stop=False)
            nc.tensor.matmul(etap[:, 0:NB - 1], cs["be127"][:],
                             phi[0:1, 1:NB], start=False, stop=False)
            nc.tensor.matmul(etap[:, 0:NB], cs["ident"][:], lam[:],
                             start=False, stop=True)
            nc.vector.tensor_copy(eta[:], etap[:, 0:NB])

            clp = spsum.tile([TS, TS], F32, tag="spA")
            nc.tensor.matmul(clp[:, 0:NB], cs["trirex"][:], eta[:],
                             start=True, stop=False)
            blkp = spsum.tile([TS, TS], F32, tag="spB")
            nc.tensor.matmul(blkp[0:1, 0:NB], cs["rcol"][:], eta[:],
                             start=True, stop=True)
            nc.vector.tensor_copy(brow[:], blkp[0:1, 0:NB])

            # 64-wide inclusive scan with ratio R2, then shift to exclusive
            for s2 in (1, 2, 4, 8, 16, 32):
                nc.vector.tensor_scalar_mul(btmp[0:1, 0:NB - s2],
                                            brow[0:1, 0:NB - s2],
                                            float(R2 ** s2))
                nc.vector.tensor_add(brow[0:1, s2:NB], brow[0:1, s2:NB],
                                     btmp[0:1, 0:NB - s2])
            nc.vector.tensor_copy(btmp[0:1, 0:NB - 1], brow[0:1, 0:NB - 1])
            nc.vector.memset(brow[0:1, 0:1], 0.0)
            nc.vector.tensor_copy(brow[0:1, 1:NB], btmp[0:1, 0:NB - 1])
            nc.vector.tensor_add(brow[:], brow[:], cs["vinrow"][:])

            nc.tensor.matmul(clp[:, 0:NB], cs["rprow"][:], brow[0:1, :],
                             start=False, stop=True)
            nc.vector.tensor_copy(cfin[:], clp[:, 0:NB])

            # delta = dt*((gp-1+R)*cfin + w3s + eta)
            k1 = float(DT * (gp - 1.0 + R))
            nc.vector.tensor_scalar_mul(delta[:], cfin[:], k1)
            nc.vector.tensor_add(lam[:], w3s[:], eta[:])   # lam reused as tmp
            nc.vector.tensor_scalar_mul(lam[:], lam[:], float(DT))
            nc.vector.tensor_add(delta[:], delta[:], lam[:])

            xlp = spsum.tile([TS, TS], F32, tag="spA")
            nc.tensor.matmul(xlp[:, 0:NB], cs["trione"][:], delta[:],
                             start=True, stop=False)
            xbp = spsum.tile([TS, TS], F32, tag="spB")
            nc.tensor.matmul(xbp[0:1, 0:NB], cs["onescol"][:], delta[:],
                             start=True, stop=True)
            nc.vector.tensor_copy(brow2[:], xbp[0:1, 0:NB])
            for s2 in (1, 2, 4, 8, 16, 32):
                nc.vector.tensor_copy(btmp[0:1, 0:NB - s2],
                                      brow2[0:1, 0:NB - s2])
                nc.vector.tensor_add(brow2[0:1, s2:NB], brow2[0:1, s2:NB],
                                     btmp[0:1, 0:NB - s2])
            nc.vector.tensor_copy(btmp[0:1, 0:NB - 1], brow2[0:1, 0:NB - 1])
            nc.vector.memset(brow2[0:1, 0:1], 0.0)
            nc.vector.tensor_copy(brow2[0:1, 1:NB], btmp[0:1, 0:NB - 1])
            nc.vector.tensor_add(brow2[:], brow2[:], cs["xinrow"][:])

            nc.tensor.matmul(xlp[:, 0:NB], cs["onesrow"][:], brow2[0:1, :],
                             start=False, stop=True)
            nc.vector.tensor_copy(xtm[:], xlp[:, 0:NB])

            # carries -> Xc row 0 (via PE transpose + SBUF->SBUF DMA)
            ctp = spsum.tile([TS, TS], F32, tag="spA")
            nc.tensor.transpose(ctp[0:NB, :], cfin[:], cs["ident"][:])
            nc.vector.tensor_copy(cfinT[:], ctp[0:NB, :])
            nc.sync.dma_start(xc[0:1, :, :], cfinT[:])

            # ---- phase C: main matmuls + interleaved output
            for g in range(NGRP):
                ot = opool.tile([TS, BPG, TS, 2], F32, tag="ot")
                for bb in range(0, BPG, 2):
                    b = g * BPG + bb
                    mpt = mpsum.tile([TS, 4 * TS], F32, tag="mpt")
                    mcat_mm = cs["mcat_r"] if use_f32r else cs["mcat"]
                    for k in range(2):
                        nc.tensor.matmul(mpt[:, 2 * TS * k:2 * TS * (k + 1)],
                                         xc[:, b + k, :], mcat_mm[:],
                                         start=True, stop=True)
                    # v (cols 0:128 of each half) -> interleaved slot 1
                    mview = mpt[:].rearrange("p (k h i) -> p k h i",
                                             k=2, h=2)
                    nc.vector.tensor_copy(ot[:, bb:bb + 2, :, 1],
                                          mview[:, :, 0, :])
                    # x (cols 128:256 of each half) + per-tile base -> slot 0
                    nc.scalar.add(ot[:, bb, :, 0], mview[:, 0, 1, :],
                                  xtm[:, b:b + 1])
                    nc.scalar.add(ot[:, bb + 1, :, 0], mview[:, 1, 1, :],
                                  xtm[:, b + 1:b + 2])
                dst = out_d[g * BPG:(g + 1) * BPG].rearrange(
                    "b p i c -> p b i c")
                nc.scalar.dma_start(dst, ot[:])

    _split_multiwaits(nc)
    return nc


# ---------------------------------------------------------------- entry point

_NC_CACHE = {}
LAST_RESULTS = None


def kernel(initial_state, actions, mass, friction_coeff):
    initial_state = np.asarray(initial_state, np.float32)
    actions = np.asarray(actions, np.float32)
    m_safe = abs(float(mass)) + 0.001
    A = 1.0 - float(friction_coeff) * DT / m_safe
    B = DT / m_safe

    F64 = actions.astype(np.float64).ravel()
    v0 = float(initial_state[0, 1])
    x0 = float(initial_state[0, 0])
    v_in, x_in = _host_carries(F64, v0, x0, A, B)
    cst, scal = _device_consts(A, B)

    key = (round(A, 15), round(B, 15))
    if key not in _NC_CACHE:
        _NC_CACHE[key] = _build_nc(scal)
    nc = _NC_CACHE[key]

    Fpad = np.empty(T, np.float32)
    Fpad[0] = 0.0
    Fpad[1:] = actions.ravel()[:-1]

    R2 = scal['R2']
    in_maps = []
    for d in range(NCORES):
        m = {"chunk": np.ascontiguousarray(
                Fpad[d * C:(d + 1) * C].reshape(NB, TS, TS))}
        m.update({k: v for k, v in cst.items()})
        m["vinrow"] = np.ascontiguousarray(
            (v_in[d] * R2 ** np.arange(NB))[None, :], np.float32)
        m["xinrow"] = np.full((1, NB), x_in[d], np.float32)
        in_maps.append(m)

    global LAST_RESULTS
    res = run_bass_kernel_spmd(nc, in_maps, core_ids=list(range(NCORES)))
    LAST_RESULTS = res

    out = np.empty((T + 1, 2), np.float32)
    for d in range(NCORES):
        out[d * C:(d + 1) * C] = res.results[d]["out"].reshape(C, 2)
    out[T, 0] = x_in[NCORES]
    out[T, 1] = v_in[NCORES]
    return out


if __name__ == "__main__":
    rng = np.random.default_rng(0)
    ins = {
        "initial_state": rng.standard_normal((1, 2)).astype(np.float32),
        "actions": rng.standard_normal((T, 1)).astype(np.float32),
        "mass": np.float32(5.0),
        "friction_coeff": np.float32(0.5),
    }
    got = kernel(**ins)
    # float64 sequential check via scipy
    from scipy.signal import lfilter
    m_safe = abs(float(ins["mass"])) + 0.001
    A = 1.0 - float(ins["friction_coeff"]) * DT / m_safe
    B = DT / m_safe
    F = ins["actions"][:, 0].astype(np.float64)
    v, _ = lfilter([B], [1, -A], F, zi=np.array([A * float(ins["initial_state"][0, 1])]))
    x = float(ins["initial_state"][0, 0]) + DT * np.cumsum(v)
    exp = np.empty((T + 1, 2))
    exp[0] = ins["initial_state"][0]
    exp[1:, 0] = x
    exp[1:, 1] = v
    rel = np.linalg.norm(got - exp) / np.linalg.norm(exp)
    print("rel err (fro) vs float64 sequential:", rel)
    print("max abs err:", np.abs(got - exp).max())


# revision 9
# speedup vs baseline: 1.1394x; 1.0721x over previous
"""Trainium2 Bass kernel for nn_DifferentiableSimulator.

Math: Euler integration of 1D kinematics is a constant-coefficient linear
recurrence  v' = A*v + B*F  (A = 1 - c*dt/m_safe, B = dt/m_safe) followed by
a cumulative sum  x' = x + dt*v'.

Sequence-parallel strategy over 8 cores (T split into 8 chunks of C=1M):
 - Host (float64): the 8 chunk-boundary states (v_in, x_in per core) via
   geometric-weight dot products — O(T) vectorized numpy, exact enough.
 - Device (per core): blocked associative scan. The chunk is cut into 8192
   tiles of 128 steps. Each tile's entire scan is ONE 128x128->[128,256]
   matmul: lhsT = Xc_block (row 0 = the tile's incoming state c_k, rows
   1..127 = forces), rhs = [Mv | Mx] constant weight matrices. Tile carries
   c_k come from a cheap hierarchical scan (per-tile weighted sums via tiny
   matmuls, 128-wide column scan via triangular matmul, 64-block row scan
   via log-step), and per-tile x offsets are added for free as the
   per-partition bias of the ScalarE activation that drains PSUM.
Everything ends in an interleaved [x, v] SBUF layout that DMAs out with 1KB
contiguous bursts.
"""
import numpy as np
import concourse.bass as bass
import concourse.mybir as mybir
import concourse.tile as tile
from concourse.bass_utils import run_bass_kernel_spmd

DT = 0.01
T = 8_388_608
NCORES = 8
C = T // NCORES          # 1,048,576 timesteps per core
NB = 64                  # blocks per core (each block = 128 tiles = 16384 steps)
TS = 128                 # tile size (steps per tile) == partitions
NGRP = 8                 # DMA pipeline groups
BPG = NB // NGRP         # blocks per group
F32 = mybir.dt.float32
F32R = mybir.dt.float32r

USE_F32R = True          # main matmuls in float32r (4x faster PE, validated)


# ---------------------------------------------------------------- host math

def _host_carries(F64, v0, x0, A, B):
    v_in = np.zeros(NCORES + 1)
    x_in = np.zeros(NCORES + 1)
    v_in[0] = v0
    x_in[0] = x0
    j = np.arange(C, dtype=np.float64)
    w_v = A ** (C - 1 - j)
    if abs(1.0 - A) > 1e-12:
        S = (1.0 - A ** (C - j)) / (1.0 - A)
        G = DT * A * (1.0 - A ** C) / (1.0 - A)
    else:
        S = C - j
        G = DT * C
    RC = A ** C
    for d in range(NCORES):
        Fc = F64[d * C:(d + 1) * C]
        x_in[d + 1] = x_in[d] + v_in[d] * G + DT * B * np.dot(S, Fc)
        v_in[d + 1] = RC * v_in[d] + B * np.dot(w_v, Fc)
    return v_in, x_in


def _device_consts(A, B):
    """Constant matrices (float64 host math, shipped as fp32)."""
    i = np.arange(TS)
    s = np.arange(TS - 1)[:, None]
    e = i[None, :] - 1 - s
    Mv = np.zeros((TS, TS))
    Mv[0, :] = A ** i
    Mv[1:, :] = np.where(e >= 0, B * A ** np.maximum(e, 0), 0.0)
    ap = A ** np.arange(TS + 1)
    cum = np.cumsum(ap)
    Mx = np.zeros((TS, TS))
    Mx[0, :] = DT * (cum[i] - 1.0)
    Mx[1:, :] = np.where(e >= 0, DT * B * cum[np.maximum(e, 0)], 0.0)
    jj = np.arange(TS)
    wlam = np.where(jj >= 1, B * A ** (128 - jj), 0.0)
    w3w = np.where(jj >= 1, B * cum[127 - jj], 0.0)
    wcat = np.stack([wlam, w3w], axis=1)
    R = A ** TS
    q = np.arange(TS)[:, None]
    m = np.arange(TS)[None, :]
    TriRex = np.where(q <= m - 1, R ** np.maximum(m - 1 - q, 0), 0.0)
    TriOnesEx = np.where(q <= m - 1, 1.0, 0.0)
    Rcol = (R ** (127 - np.arange(TS)))[:, None]
    OnesCol = np.ones((TS, 1))
    Rprow = (R ** np.arange(TS))[None, :]
    OnesRow = np.ones((1, TS))
    Bsub = np.zeros((TS, TS))
    Bsub[np.arange(1, TS), np.arange(TS - 1)] = B
    Be127 = np.zeros((1, TS))
    Be127[0, 127] = B
    Ident = np.eye(TS)
    cst = dict(mcat=np.concatenate([Mv, Mx], axis=1), wcat=wcat,
               trirex=TriRex, trione=TriOnesEx, rcol=Rcol, onescol=OnesCol,
               rprow=Rprow, onesrow=OnesRow, bsub=Bsub, be127=Be127,
               ident=Ident)
    cst = {k: np.ascontiguousarray(v, np.float32) for k, v in cst.items()}
    scal = dict(R=R, R2=R ** TS, gp=float(cum[127]))
    return cst, scal


# ---------------------------------------------------------------- bass build

def _split_multiwaits(nc, maxw=1):
    """The walrus build in this container accepts at most ONE sync wait per
    instruction (Tile's scheduler happily attaches several). Keep the first
    wait on the instruction; move extras onto EventSemaphore instructions
    inserted immediately before, same engine."""
    n = 0
    for fn in nc.m.functions:
        for bb in fn.blocks:
            new_insts = []
            changed = False
            for inst in bb.instructions:
                si = inst.sync_info
                waits = list(si.on_wait) if si and si.on_wait else []
                if len(waits) > maxw:
                    changed = True
                    for w in waits[maxw:]:
                        ev = mybir.InstEventSemaphore(
                            name=f"{inst.name}-ws{n}", engine=inst.engine,
                            ins=[], outs=[],
                            sync_info=mybir.SyncInfo(on_wait=[w], on_update=[]),
                        )
                        new_insts.append(ev)
                        n += 1
                    si.on_wait = waits[:maxw]
                new_insts.append(inst)
            if changed:
                bb.instructions = new_insts
    return n


def _build_nc(scal, use_f32r=USE_F32R):
    R2 = scal['R2']
    gp = scal['gp']
    R = scal['R']
    nc = bass.Bass("TRN2", target_bir_lowering=False, debug=False)

    chunk = nc.dram_tensor("chunk", [NB, TS, TS], F32, kind="ExternalInput")
    out_d = nc.dram_tensor("out", [NB, TS, TS, 2], F32, kind="ExternalOutput")
    cd = {}
    for name, shape in [("mcat", [TS, 2 * TS]), ("wcat", [TS, 2]),
                        ("trirex", [TS, TS]), ("trione", [TS, TS]),
                        ("rcol", [TS, 1]), ("onescol", [TS, 1]),
                        ("rprow", [1, TS]), ("onesrow", [1, TS]),
                        ("bsub", [TS, TS]), ("be127", [1, TS]),
                        ("ident", [TS, TS]),
                        ("vinrow", [1, NB]), ("xinrow", [1, NB])]:
        cd[name] = nc.dram_tensor(name, shape, F32, kind="ExternalInput")

    with tile.TileContext(nc) as tc:
        with (
            tc.tile_pool(name="const", bufs=1) as cpool,
            tc.tile_pool(name="pers", bufs=1) as pers,
            tc.tile_pool(name="yp", bufs=3) as ypool,
            tc.tile_pool(name="op", bufs=2) as opool,
            tc.tile_pool(name="tp", bufs=2, space="PSUM") as tpsum,
            tc.tile_pool(name="mp", bufs=3, space="PSUM") as mpsum,
            tc.tile_pool(name="sp", bufs=1, space="PSUM") as spsum,
            tc.tile_pool(name="lw", bufs=1, space="PSUM") as lwpsum,
        ):
            # ---- constants to SBUF
            cs = {}
            for name, t in cd.items():
                ct = cpool.tile(list(t.shape), F32, tag=f"c_{name}")
                nc.gpsimd.dma_start(ct[:], t[:])
                cs[name] = ct
            if use_f32r:
                # rounded copies for fp32r matmul operands
                for name in ("mcat", "wcat"):
                    rt = cpool.tile(list(cd[name].shape), F32R,
                                    tag=f"r_{name}")
                    nc.vector.tensor_copy(rt[:], cs[name][:])
                    cs[name + "_r"] = rt

            # ---- persistent tiles
            xc = pers.tile([TS, NB, TS], F32R if use_f32r else F32, tag="xc")       # [j', b, p]
            phi = pers.tile([TS, NB], F32, tag="phi")
            lam = pers.tile([TS, NB], F32, tag="lam")
            w3s = pers.tile([TS, NB], F32, tag="w3s")
            eta = pers.tile([TS, NB], F32, tag="eta")
            cfin = pers.tile([TS, NB], F32, tag="cfin")
            xtm = pers.tile([TS, NB], F32, tag="xtm")
            delta = pers.tile([TS, NB], F32, tag="delta")
            cfinT = pers.tile([NB, TS], F32R if use_f32r else F32, tag="cfinT")
            brow = pers.tile([1, NB], F32, tag="brow")
            brow2 = pers.tile([1, NB], F32, tag="brow2")
            btmp = pers.tile([1, NB], F32, tag="btmp")

            lamw3 = lwpsum.tile([TS, TS], F32, tag="lamw3")   # cols 2b, 2b+1

            # ---- phase A: load, transpose to Xc (dense on PE), then
            # phase-1 matmuls (dense, after copies land)
            for g in range(NGRP):
                yt = ypool.tile([TS, BPG, TS], F32, tag="yt")
                src = chunk[g * BPG:(g + 1) * BPG].rearrange("b p j -> p b j")
                nc.sync.dma_start(yt[:], src)
                # phi slice: j'=0 of each block
                nc.vector.tensor_copy(phi[:, g * BPG:(g + 1) * BPG],
                                      yt[:, :, 0])
                for bb in range(0, BPG, 2):
                    b = g * BPG + bb
                    tpt = tpsum.tile([TS, 2 * TS], F32, tag="tpt")
                    nc.tensor.transpose(tpt[:, 0:TS], yt[:, bb, :],
                                        cs["ident"][:])
                    nc.tensor.transpose(tpt[:, TS:2 * TS], yt[:, bb + 1, :],
                                        cs["ident"][:])
                    eng = nc.vector if (bb // 2) % 2 == 0 else nc.scalar
                    if eng is nc.vector:
                        eng.tensor_copy(xc[:, b:b + 2, :], tpt[:])
                    else:
                        eng.copy(xc[:, b:b + 2, :], tpt[:])
            wcat_mm = cs["wcat_r"] if use_f32r else cs["wcat"]
            for b in range(NB):
                # phase-1: per-tile weighted sums (weights 0 at j'=0)
                nc.tensor.matmul(lamw3[:, 2 * b:2 * b + 2],
                                 xc[:, b, :], wcat_mm[:],
                                 start=True, stop=True)

            # ---- phase B: tiny hierarchical scan for carries
            lw2 = lamw3[:].rearrange("p (b two) -> p b two", two=2)
            nc.vector.tensor_copy(lam[:], lw2[:, :, 0])
            nc.vector.tensor_copy(w3s[:], lw2[:, :, 1])

            etap = spsum.tile([TS, TS], F32, tag="spA")
            nc.tensor.matmul(etap[:, 0:NB], cs["bsub"][:], phi[:],
                             start=True, stop=False)
            nc.tensor.matmul(etap[:, 0:NB - 1], cs["be127"][:],
                             phi[0:1, 1:NB], start=False, stop=False)
            nc.tensor.matmul(etap[:, 0:NB], cs["ident"][:], lam[:],
                             start=False, stop=True)
            nc.vector.tensor_copy(eta[:], etap[:, 0:NB])

            clp = spsum.tile([TS, TS], F32, tag="spA")
            nc.tensor.matmul(clp[:, 0:NB], cs["trirex"][:], eta[:],
                             start=True, stop=False)
            blkp = spsum.tile([TS, TS], F32, tag="spB")
            nc.tensor.matmul(blkp[0:1, 0:NB], cs["rcol"][:], eta[:],
                             start=True, stop=True)
            nc.vector.tensor_copy(brow[:], blkp[0:1, 0:NB])

            # 64-wide inclusive scan with ratio R2, then shift to exclusive
            for s2 in (1, 2, 4, 8, 16, 32):
                nc.vector.tensor_scalar_mul(btmp[0:1, 0:NB - s2],
                                            brow[0:1, 0:NB - s2],
                                            float(R2 ** s2))
                nc.vector.tensor_add(brow[0:1, s2:NB], brow[0:1, s2:NB],
                                     btmp[0:1, 0:NB - s2])
            nc.vector.tensor_copy(btmp[0:1, 0:NB - 1], brow[0:1, 0:NB - 1])
            nc.vector.tensor_copy(brow[0:1, 0:1], cs["vinrow"][0:1, 0:1])
            nc.vector.tensor_add(brow[0:1, 1:NB], btmp[0:1, 0:NB - 1],
                                 cs["vinrow"][0:1, 1:NB])

            nc.tensor.matmul(clp[:, 0:NB], cs["rprow"][:], brow[0:1, :],
                             start=False, stop=True)
            nc.vector.tensor_copy(cfin[:], clp[:, 0:NB])

            # delta = dt*((gp-1+R)*cfin + w3s + eta)
            k1 = float(DT * (gp - 1.0 + R))
            nc.vector.tensor_scalar_mul(delta[:], cfin[:], k1)
            nc.vector.tensor_add(lam[:], w3s[:], eta[:])   # lam reused as tmp
            nc.vector.tensor_scalar_mul(lam[:], lam[:], float(DT))
            nc.vector.tensor_add(delta[:], delta[:], lam[:])

            xlp = spsum.tile([TS, TS], F32, tag="spA")
            nc.tensor.matmul(xlp[:, 0:NB], cs["trione"][:], delta[:],
                             start=True, stop=False)
            xbp = spsum.tile([TS, TS], F32, tag="spB")
            nc.tensor.matmul(xbp[0:1, 0:NB], cs["onescol"][:], delta[:],
                             start=True, stop=True)
            nc.vector.tensor_copy(brow2[:], xbp[0:1, 0:NB])
            for s2 in (1, 2, 4, 8, 16, 32):
                nc.vector.tensor_copy(btmp[0:1, 0:NB - s2],
                                      brow2[0:1, 0:NB - s2])
                nc.vector.tensor_add(brow2[0:1, s2:NB], brow2[0:1, s2:NB],
                                     btmp[0:1, 0:NB - s2])
            nc.vector.tensor_copy(btmp[0:1, 0:NB - 1], brow2[0:1, 0:NB - 1])
            nc.vector.tensor_copy(brow2[0:1, 0:1], cs["xinrow"][0:1, 0:1])
            nc.vector.tensor_add(brow2[0:1, 1:NB], btmp[0:1, 0:NB - 1],
                                 cs["xinrow"][0:1, 1:NB])

            nc.tensor.matmul(xlp[:, 0:NB], cs["onesrow"][:], brow2[0:1, :],
                             start=False, stop=True)
            nc.vector.tensor_copy(xtm[:], xlp[:, 0:NB])

            # carries -> Xc row 0 (via PE transpose + SBUF->SBUF DMA)
            ctp = spsum.tile([TS, TS], F32, tag="spA")
            nc.tensor.transpose(ctp[0:NB, :], cfin[:], cs["ident"][:])
            nc.vector.tensor_copy(cfinT[:], ctp[0:NB, :])
            nc.sync.dma_start(xc[0:1, :, :], cfinT[:])

            # ---- phase C: main matmuls + interleaved output
            for g in range(NGRP):
                ot = opool.tile([TS, BPG, TS, 2], F32, tag="ot")
                for bb in range(0, BPG, 2):
                    b = g * BPG + bb
                    mpt = mpsum.tile([TS, 4 * TS], F32, tag="mpt")
                    mcat_mm = cs["mcat_r"] if use_f32r else cs["mcat"]
                    for k in range(2):
                        nc.tensor.matmul(mpt[:, 2 * TS * k:2 * TS * (k + 1)],
                                         xc[:, b + k, :], mcat_mm[:],
                                         start=True, stop=True)
                    # v (cols 0:128 of each half) -> interleaved slot 1
                    mview = mpt[:].rearrange("p (k h i) -> p k h i",
                                             k=2, h=2)
                    pair = bb // 2
                    if pair % 2 == 0:
                        nc.vector.tensor_copy(ot[:, bb:bb + 2, :, 1],
                                              mview[:, :, 0, :])
                        nc.scalar.add(ot[:, bb, :, 0], mview[:, 0, 1, :],
                                      xtm[:, b:b + 1])
                        nc.vector.tensor_scalar(
                            ot[:, bb + 1, :, 0], mview[:, 1, 1, :],
                            xtm[:, b + 1:b + 2], None,
                            op0=mybir.AluOpType.add)
                    else:
                        nc.scalar.copy(ot[:, bb:bb + 2, :, 1],
                                       mview[:, :, 0, :])
                        nc.vector.tensor_scalar(
                            ot[:, bb, :, 0], mview[:, 0, 1, :],
                            xtm[:, b:b + 1], None,
                            op0=mybir.AluOpType.add)
                        nc.scalar.add(ot[:, bb + 1, :, 0], mview[:, 1, 1, :],
                                      xtm[:, b + 1:b + 2])
                dst = out_d[g * BPG:(g + 1) * BPG].rearrange(
                    "b p i c -> p b i c")
                nc.scalar.dma_start(dst, ot[:])

    _split_multiwaits(nc)
    return nc


# ---------------------------------------------------------------- entry point

_NC_CACHE = {}
LAST_RESULTS = None


def kernel(initial_state, actions, mass, friction_coeff):
    initial_state = np.asarray(initial_state, np.float32)
    actions = np.asarray(actions, np.float32)
    m_safe = abs(float(mass)) + 0.001
    A = 1.0 - float(friction_coeff) * DT / m_safe
    B = DT / m_safe

    F64 = actions.astype(np.float64).ravel()
    v0 = float(initial_state[0, 1])
    x0 = float(initial_state[0, 0])
    v_in, x_in = _host_carries(F64, v0, x0, A, B)
    cst, scal = _device_consts(A, B)

    key = (round(A, 15), round(B, 15))
    if key not in _NC_CACHE:
        _NC_CACHE[key] = _build_nc(scal)
    nc = _NC_CACHE[key]

    Fpad = np.empty(T, np.float32)
    Fpad[0] = 0.0
    Fpad[1:] = actions.ravel()[:-1]

    R2 = scal['R2']
    in_maps = []
    for d in range(NCORES):
        m = {"chunk": np.ascontiguousarray(
                Fpad[d * C:(d + 1) * C].reshape(NB, TS, TS))}
        m.update({k: v for k, v in cst.items()})
        m["vinrow"] = np.ascontiguousarray(
            (v_in[d] * R2 ** np.arange(NB))[None, :], np.float32)
        m["xinrow"] = np.full((1, NB), x_in[d], np.float32)
        in_maps.append(m)

    global LAST_RESULTS
    res = run_bass_kernel_spmd(nc, in_maps, core_ids=list(range(NCORES)))
    LAST_RESULTS = res

    out = np.empty((T + 1, 2), np.float32)
    for d in range(NCORES):
        out[d * C:(d + 1) * C] = res.results[d]["out"].reshape(C, 2)
    out[T, 0] = x_in[NCORES]
    out[T, 1] = v_in[NCORES]
    return out


if __name__ == "__main__":
    rng = np.random.default_rng(0)
    ins = {
        "initial_state": rng.standard_normal((1, 2)).astype(np.float32),
        "actions": rng.standard_normal((T, 1)).astype(np.float32),
        "mass": np.float32(5.0),
        "friction_coeff": np.float32(0.5),
    }
    got = kernel(**ins)
    # float64 sequential check via scipy
    from scipy.signal import lfilter
    m_safe = abs(float(ins["mass"])) + 0.001
    A = 1.0 - float(ins["friction_coeff"]) * DT / m_safe
    B = DT / m_safe
    F = ins["actions"][:, 0].astype(np.float64)
    v, _ = lfilter([B], [1, -A], F, zi=np.array([A * float(ins["initial_state"][0, 1])]))
    x = float(ins["initial_state"][0, 0]) + DT * np.cumsum(v)
    exp = np.empty((T + 1, 2))
    exp[0] = ins["initial_state"][0]
    exp[1:, 0] = x
    exp[1:, 1] = v
    rel = np.linalg.norm(got - exp) / np.linalg.norm(exp)
    print("rel err (fro) vs float64 sequential:", rel)
    print("max abs err:", np.abs(got - exp).max())


# revision 10
# speedup vs baseline: 1.5153x; 1.3299x over previous
"""Trainium2 Bass kernel for nn_DifferentiableSimulator.

Math: Euler integration of 1D kinematics is a constant-coefficient linear
recurrence  v' = A*v + B*F  (A = 1 - c*dt/m_safe, B = dt/m_safe) followed by
a cumulative sum  x' = x + dt*v'.

Sequence-parallel strategy over 8 cores (T split into 8 chunks of C=1M):
 - Host (float64): the 8 chunk-boundary states (v_in, x_in per core) via
   geometric-weight dot products, plus pure layout prep: each chunk is
   handed to its core pre-transposed as Xc[j', k] = F[dC + 128k + j' - 1]
   (128 partitions = within-tile position, 8192 columns = tiles).
 - Device (per core): blocked associative scan over 8192 tiles of 128
   steps. Phase 1 computes per-tile weighted sums with one tiny matmul per
   128-tile block; a cheap hierarchical scan (triangular-matrix matmuls
   for the 128-wide column scan, log-step for the 64-block row scan)
   produces every tile's incoming state c_k and position base xb_k. The
   carries are DMA'd into row 0 of Xc, and the whole v/x trajectory then
   falls out of matmuls with CONSTANT stationary weights: out_v = Mv @ Xc,
   out_x = Mx @ Xc + ones ⊗ xbrow, streamed 512 columns at a time (fp32r).
 - Output: planar [i, k] fp32 planes per core; host re-interleaves to the
   [T+1, 2] result (pure layout again).
"""
import numpy as np
import concourse.bass as bass
import concourse.mybir as mybir
import concourse.tile as tile
from concourse.bass_utils import run_bass_kernel_spmd

DT = 0.01
T = 8_388_608
NCORES = 8
C = T // NCORES          # 1,048,576 timesteps per core
NT = C // 128            # 8192 tiles per core
NB = 64                  # blocks (of 128 tiles) per core
TS = 128
NGRP = 8                 # load pipeline groups
CPG = NT // NGRP         # 1024 tile-columns per load group
NCH = NT // 512          # 16 main-matmul chunks of 512 columns
F32 = mybir.dt.float32
F32R = mybir.dt.float32r

USE_F32R = True


# ---------------------------------------------------------------- host math

def _host_carries(F64, v0, x0, A, B):
    v_in = np.zeros(NCORES + 1)
    x_in = np.zeros(NCORES + 1)
    v_in[0] = v0
    x_in[0] = x0
    j = np.arange(C, dtype=np.float64)
    w_v = A ** (C - 1 - j)
    if abs(1.0 - A) > 1e-12:
        S = (1.0 - A ** (C - j)) / (1.0 - A)
        G = DT * A * (1.0 - A ** C) / (1.0 - A)
    else:
        S = C - j
        G = DT * C
    RC = A ** C
    for d in range(NCORES):
        Fc = F64[d * C:(d + 1) * C]
        x_in[d + 1] = x_in[d] + v_in[d] * G + DT * B * np.dot(S, Fc)
        v_in[d + 1] = RC * v_in[d] + B * np.dot(w_v, Fc)
    return v_in, x_in


def _device_consts(A, B):
    """Constant matrices (float64 host math, shipped as fp32)."""
    i = np.arange(TS)
    s = np.arange(TS - 1)[:, None]
    e = i[None, :] - 1 - s
    Mv = np.zeros((TS, TS))
    Mv[0, :] = A ** i
    Mv[1:, :] = np.where(e >= 0, B * A ** np.maximum(e, 0), 0.0)
    ap = A ** np.arange(TS + 1)
    cum = np.cumsum(ap)
    Mx = np.zeros((TS, TS))
    Mx[0, :] = DT * (cum[i] - 1.0)
    Mx[1:, :] = np.where(e >= 0, DT * B * cum[np.maximum(e, 0)], 0.0)
    jj = np.arange(TS)
    wlam = np.where(jj >= 1, B * A ** (128 - jj), 0.0)
    w3w = np.where(jj >= 1, B * cum[127 - jj], 0.0)
    wcat = np.stack([wlam, w3w], axis=1)
    R = A ** TS
    q = np.arange(TS)[:, None]
    m = np.arange(TS)[None, :]
    TriRex = np.where(q <= m - 1, R ** np.maximum(m - 1 - q, 0), 0.0)
    TriOnesEx = np.where(q <= m - 1, 1.0, 0.0)
    Rcol = (R ** (127 - np.arange(TS)))[:, None]
    OnesCol = np.ones((TS, 1))
    Rprow = (R ** np.arange(TS))[None, :]
    OnesRow = np.ones((1, TS))
    Bsub = np.zeros((TS, TS))
    Bsub[np.arange(1, TS), np.arange(TS - 1)] = B
    Be127 = np.zeros((1, TS))
    Be127[0, 127] = B
    Ident = np.eye(TS)
    cst = dict(mv=Mv, mx=Mx, wcat=wcat,
               trirex=TriRex, trione=TriOnesEx, rcol=Rcol, onescol=OnesCol,
               rprow=Rprow, onesrow=OnesRow, bsub=Bsub, be127=Be127,
               ident=Ident)
    cst = {k: np.ascontiguousarray(v, np.float32) for k, v in cst.items()}
    scal = dict(R=R, R2=R ** TS, gp=float(cum[127]))
    return cst, scal


# ---------------------------------------------------------------- bass build

def _split_multiwaits(nc, maxw=1):
    """The walrus build in this container accepts at most ONE sync wait per
    instruction (Tile's scheduler happily attaches several). Keep the first
    wait on the instruction; move extras onto EventSemaphore instructions
    inserted immediately before, same engine."""
    n = 0
    for fn in nc.m.functions:
        for bb in fn.blocks:
            new_insts = []
            changed = False
            for inst in bb.instructions:
                si = inst.sync_info
                waits = list(si.on_wait) if si and si.on_wait else []
                if len(waits) > maxw:
                    changed = True
                    for w in waits[maxw:]:
                        ev = mybir.InstEventSemaphore(
                            name=f"{inst.name}-ws{n}", engine=inst.engine,
                            ins=[], outs=[],
                            sync_info=mybir.SyncInfo(on_wait=[w], on_update=[]),
                        )
                        new_insts.append(ev)
                        n += 1
                    si.on_wait = waits[:maxw]
                new_insts.append(inst)
            if changed:
                bb.instructions = new_insts
    return n


def _build_nc(scal, use_f32r=USE_F32R):
    R2 = scal['R2']
    gp = scal['gp']
    R = scal['R']
    FD = F32R if use_f32r else F32
    nc = bass.Bass("TRN2", target_bir_lowering=False, debug=False)

    chunk = nc.dram_tensor("chunk", [TS, NT], FD, kind="ExternalInput")
    phi_d = nc.dram_tensor("phi", [TS, NB], F32, kind="ExternalInput")
    outv_d = nc.dram_tensor("outv", [TS, NT], F32, kind="ExternalOutput")
    outx_d = nc.dram_tensor("outx", [TS, NT], F32, kind="ExternalOutput")
    cd = {}
    for name, shape, dt_ in [
            ("mv", [TS, TS], FD), ("mx", [TS, TS], FD),
            ("wcat", [TS, 2], FD),
            ("trirex", [TS, TS], F32), ("trione", [TS, TS], F32),
            ("rcol", [TS, 1], F32), ("onescol", [TS, 1], F32),
            ("rprow", [1, TS], F32), ("onesrow", [1, TS], FD),
            ("bsub", [TS, TS], F32), ("be127", [1, TS], F32),
            ("ident", [TS, TS], F32),
            ("vinrow", [1, NB], F32), ("xinrow", [1, NB], F32)]:
        cd[name] = nc.dram_tensor(name, shape, dt_, kind="ExternalInput")

    with tile.TileContext(nc) as tc:
        with (
            tc.tile_pool(name="const", bufs=1) as cpool,
            tc.tile_pool(name="pers", bufs=1) as pers,
            tc.tile_pool(name="ov", bufs=1) as ovpool,
            tc.tile_pool(name="mp", bufs=5, space="PSUM") as mpsum,
            tc.tile_pool(name="sp", bufs=1, space="PSUM") as spsum,
            tc.tile_pool(name="lw", bufs=1, space="PSUM") as lwpsum,
        ):
            cs = {}
            for name, t in cd.items():
                ct = cpool.tile(list(t.shape), t.dtype, tag=f"c_{name}")
                nc.gpsimd.dma_start(ct[:], t[:])
                cs[name] = ct

            xc = pers.tile([TS, NT], FD, tag="xc")            # [j', k]
            phi = pers.tile([TS, NB], F32, tag="phi")
            lam = pers.tile([TS, NB], F32, tag="lam")
            w3s = pers.tile([TS, NB], F32, tag="w3s")
            eta = pers.tile([TS, NB], F32, tag="eta")
            cfin = pers.tile([TS, NB], F32, tag="cfin")
            xtm = pers.tile([TS, NB], F32, tag="xtm")
            delta = pers.tile([TS, NB], F32, tag="delta")
            cfinT = pers.tile([NB, TS], FD, tag="cfinT")
            xtmT = pers.tile([NB, TS], FD, tag="xtmT")
            xbrow = pers.tile([1, NT], FD, tag="xbrow")
            brow = pers.tile([1, NB], F32, tag="brow")
            brow2 = pers.tile([1, NB], F32, tag="brow2")
            btmp = pers.tile([1, NB], F32, tag="btmp")
            outv = ovpool.tile([TS, NT], F32, tag="outv")
            outx = ovpool.tile([TS, NT], F32, tag="outx")

            nc.gpsimd.dma_start(phi[:], phi_d[:])

            lamw3 = lwpsum.tile([TS, TS], F32, tag="lamw3")

            # ---- phase A: stream Xc in; per-block phase-1 matmuls
            for g in range(NGRP):
                nc.sync.dma_start(xc[:, g * CPG:(g + 1) * CPG],
                                  chunk[:, g * CPG:(g + 1) * CPG])
                for bb in range(CPG // TS):
                    b = g * (CPG // TS) + bb
                    nc.tensor.matmul(lamw3[:, 2 * b:2 * b + 2],
                                     xc[:, b * TS:(b + 1) * TS],
                                     cs["wcat"][:], start=True, stop=True)

            # ---- phase B: tiny hierarchical scan for carries
            lw2 = lamw3[:].rearrange("p (b two) -> p b two", two=2)
            nc.vector.tensor_copy(lam[:], lw2[:, :, 0])
            nc.vector.tensor_copy(w3s[:], lw2[:, :, 1])

            etap = spsum.tile([TS, TS], F32, tag="spA")
            nc.tensor.matmul(etap[:, 0:NB], cs["bsub"][:], phi[:],
                             start=True, stop=False)
            nc.tensor.matmul(etap[:, 0:NB - 1], cs["be127"][:],
                             phi[0:1, 1:NB], start=False, stop=True)
            nc.vector.tensor_add(eta[:], etap[:, 0:NB], lam[:])

            clp = spsum.tile([TS, TS], F32, tag="spA")
            nc.tensor.matmul(clp[:, 0:NB], cs["trirex"][:], eta[:],
                             start=True, stop=False)
            blkp = spsum.tile([TS, TS], F32, tag="spB")
            nc.tensor.matmul(blkp[0:1, 0:NB], cs["rcol"][:], eta[:],
                             start=True, stop=True)
            nc.vector.tensor_copy(brow[:], blkp[0:1, 0:NB])

            for s2 in (1, 2, 4, 8, 16, 32):
                nc.vector.tensor_scalar_mul(btmp[0:1, 0:NB - s2],
                                            brow[0:1, 0:NB - s2],
                                            float(R2 ** s2))
                nc.vector.tensor_add(brow[0:1, s2:NB], brow[0:1, s2:NB],
                                     btmp[0:1, 0:NB - s2])
            nc.vector.tensor_copy(btmp[0:1, 0:NB - 1], brow[0:1, 0:NB - 1])
            nc.vector.tensor_copy(brow[0:1, 0:1], cs["vinrow"][0:1, 0:1])
            nc.vector.tensor_add(brow[0:1, 1:NB], btmp[0:1, 0:NB - 1],
                                 cs["vinrow"][0:1, 1:NB])

            nc.tensor.matmul(clp[:, 0:NB], cs["rprow"][:], brow[0:1, :],
                             start=False, stop=True)
            nc.vector.tensor_copy(cfin[:], clp[:, 0:NB])

            k1 = float(DT * (gp - 1.0 + R))
            nc.vector.tensor_scalar_mul(delta[:], cfin[:], k1)
            nc.vector.tensor_add(lam[:], w3s[:], eta[:])   # lam reused as tmp
            nc.vector.tensor_scalar_mul(lam[:], lam[:], float(DT))
            nc.vector.tensor_add(delta[:], delta[:], lam[:])

            xlp = spsum.tile([TS, TS], F32, tag="spA")
            nc.tensor.matmul(xlp[:, 0:NB], cs["trione"][:], delta[:],
                             start=True, stop=False)
            xbp = spsum.tile([TS, TS], F32, tag="spB")
            nc.tensor.matmul(xbp[0:1, 0:NB], cs["onescol"][:], delta[:],
                             start=True, stop=True)
            nc.vector.tensor_copy(brow2[:], xbp[0:1, 0:NB])
            for s2 in (1, 2, 4, 8, 16, 32):
                nc.vector.tensor_copy(btmp[0:1, 0:NB - s2],
                                      brow2[0:1, 0:NB - s2])
                nc.vector.tensor_add(brow2[0:1, s2:NB], brow2[0:1, s2:NB],
                                     btmp[0:1, 0:NB - s2])
            nc.vector.tensor_copy(btmp[0:1, 0:NB - 1], brow2[0:1, 0:NB - 1])
            nc.vector.tensor_copy(brow2[0:1, 0:1], cs["xinrow"][0:1, 0:1])
            nc.vector.tensor_add(brow2[0:1, 1:NB], btmp[0:1, 0:NB - 1],
                                 cs["xinrow"][0:1, 1:NB])

            nc.tensor.matmul(xlp[:, 0:NB], cs["onesrow"][:].bitcast(F32),
                             brow2[0:1, :], start=False, stop=True)
            nc.vector.tensor_copy(xtm[:], xlp[:, 0:NB])

            # carries -> Xc row 0 ; per-tile x bases -> xbrow [1, NT]
            ctp = spsum.tile([TS, TS], F32, tag="spA")
            nc.tensor.transpose(ctp[0:NB, :], cfin[:], cs["ident"][:])
            nc.vector.tensor_copy(cfinT[:], ctp[0:NB, :])
            nc.sync.dma_start(xc[0:1, :], cfinT[:])
            xtp = spsum.tile([TS, TS], F32, tag="spA")
            nc.tensor.transpose(xtp[0:NB, :], xtm[:], cs["ident"][:])
            nc.vector.tensor_copy(xtmT[:], xtp[0:NB, :])
            nc.sync.dma_start(xbrow[0:1, :], xtmT[:])

            # ---- phase C: constant-weight main matmuls, planar drains
            half = NCH // 2   # 2 rounds of 8 chunks (PSUM bank limit)
            for r in range(2):
                vps = []
                for cch in range(r * half, (r + 1) * half):
                    vp = mpsum.tile([TS, 512], F32, tag="mpt")
                    nc.tensor.matmul(vp[:], cs["mv"][:],
                                     xc[:, cch * 512:(cch + 1) * 512],
                                     start=True, stop=True)
                    vps.append((cch, vp))
                for idx, (cch, vp) in enumerate(vps):
                    sl = outv[:, cch * 512:(cch + 1) * 512]
                    if idx % 2 == 0:
                        nc.vector.tensor_copy(sl, vp[:])
                    else:
                        nc.scalar.copy(sl, vp[:])
                xps = []
                for cch in range(r * half, (r + 1) * half):
                    xp = mpsum.tile([TS, 512], F32, tag="mpt")
                    nc.tensor.matmul(xp[:], cs["mx"][:],
                                     xc[:, cch * 512:(cch + 1) * 512],
                                     start=True, stop=False)
                    xps.append((cch, xp))
                for cch, xp in xps:
                    nc.tensor.matmul(xp[:], cs["onesrow"][:],
                                     xbrow[0:1, cch * 512:(cch + 1) * 512],
                                     start=False, stop=True)
                for idx, (cch, xp) in enumerate(xps):
                    sl = outx[:, cch * 512:(cch + 1) * 512]
                    if idx % 2 == 0:
                        nc.scalar.copy(sl, xp[:])
                    else:
                        nc.vector.tensor_copy(sl, xp[:])
                nc.sync.dma_start(
                    outv_d[:, r * half * 512:(r + 1) * half * 512],
                    outv[:, r * half * 512:(r + 1) * half * 512])
                nc.scalar.dma_start(
                    outx_d[:, r * half * 512:(r + 1) * half * 512],
                    outx[:, r * half * 512:(r + 1) * half * 512])

    _split_multiwaits(nc)
    return nc


# ---------------------------------------------------------------- entry point

_NC_CACHE = {}
LAST_RESULTS = None


def kernel(initial_state, actions, mass, friction_coeff):
    global LAST_RESULTS
    initial_state = np.asarray(initial_state, np.float32)
    actions = np.asarray(actions, np.float32)
    m_safe = abs(float(mass)) + 0.001
    A = 1.0 - float(friction_coeff) * DT / m_safe
    B = DT / m_safe

    F64 = actions.astype(np.float64).ravel()
    v0 = float(initial_state[0, 1])
    x0 = float(initial_state[0, 0])
    v_in, x_in = _host_carries(F64, v0, x0, A, B)
    cst, scal = _device_consts(A, B)

    key = (round(A, 15), round(B, 15))
    if key not in _NC_CACHE:
        _NC_CACHE[key] = _build_nc(scal)
    nc = _NC_CACHE[key]

    Fpad = np.empty(T, np.float32)
    Fpad[0] = 0.0
    Fpad[1:] = actions.ravel()[:-1]

    R2 = scal['R2']
    in_maps = []
    for d in range(NCORES):
        ch = Fpad[d * C:(d + 1) * C].reshape(NT, TS)
        m = {"chunk": np.ascontiguousarray(ch.T),        # [j', k]
             "phi": np.ascontiguousarray(
                 ch[:, 0].reshape(NB, TS).T)}            # [p, b]
        m.update({k: v for k, v in cst.items()})
        m["vinrow"] = np.ascontiguousarray(
            (v_in[d] * R2 ** np.arange(NB))[None, :], np.float32)
        m["xinrow"] = np.full((1, NB), x_in[d], np.float32)
        in_maps.append(m)

    res = run_bass_kernel_spmd(nc, in_maps, core_ids=list(range(NCORES)))
    LAST_RESULTS = res

    out = np.empty((T + 1, 2), np.float32)
    for d in range(NCORES):
        sl = out[d * C:(d + 1) * C]
        sl[:, 0] = res.results[d]["outx"].T.ravel()
        sl[:, 1] = res.results[d]["outv"].T.ravel()
    out[T, 0] = x_in[NCORES]
    out[T, 1] = v_in[NCORES]
    return out


if __name__ == "__main__":
    rng = np.random.default_rng(0)
    ins = {
        "initial_state": rng.standard_normal((1, 2)).astype(np.float32),
        "actions": rng.standard_normal((T, 1)).astype(np.float32),
        "mass": np.float32(5.0),
        "friction_coeff": np.float32(0.5),
    }
    got = kernel(**ins)
    from scipy.signal import lfilter
    m_safe = abs(float(ins["mass"])) + 0.001
    A = 1.0 - float(ins["friction_coeff"]) * DT / m_safe
    B = DT / m_safe
    F = ins["actions"][:, 0].astype(np.float64)
    v, _ = lfilter([B], [1, -A], F,
                   zi=np.array([A * float(ins["initial_state"][0, 1])]))
    x = float(ins["initial_state"][0, 0]) + DT * np.cumsum(v)
    exp = np.empty((T + 1, 2))
    exp[0] = ins["initial_state"][0]
    exp[1:, 0] = x
    exp[1:, 1] = v
    rel = np.linalg.norm(got - exp) / np.linalg.norm(exp)
    print("rel err (fro) vs float64 sequential:", rel)
    print("max abs err:", np.abs(got - exp).max())
